# revision 7
# baseline (speedup 1.0000x reference)
"""DigitCaps dynamic-routing kernel for 8x Trainium2 NeuronCores.

Full inputs -> batch-sharded across 8 cores (16 samples/core), W replicated.

Per-core layout:
  u_hat[(r_l,b)=128 partitions, g=256, o=16, c=10]  (bf16 in SBUF)
    where route r = g*8 + r_l   (8 routes per matmul group)
  b_ij / c_ij: (128p, 256g, 10c)   (partition carries (r_l, b))

u_hat build: per group g, one PE matmul:
  lhsT = Xblk[g] (K=64=(r_l,i), M=128=(r_l,b))  block-diagonal x (host-built)
  rhs  = Wt[g]   (K=64=(r_l,i), N=160=(o,c))
  out  = psum (128=(r_l,b), 160=(o,c)) -> drain to SBUF as bf16

s_j = sum_r c*u_hat: PE matmuls with constant lhsT = tile(eye(16),(8,8)):
  sums over partitions (r_l) while replicating the (16b, 160) result to all
  128 partitions; groups are summed 3-at-a-time into 3 psum slots
  (accumulated over chunks) + a final 2-op add.

agreement = sum_o u_hat*v: DVE broadcast multiply + tree-reduce over o.
softmax over c: free-dim innermost; squash: tiny (128,160) ops.
"""

import sys

for p in ("/opt/trn_rl_repo",):
    if p not in sys.path:
        sys.path.insert(0, p)

import numpy as np

import concourse.bass as bass
import concourse.bacc as bacc
import concourse.mybir as mybir
import concourse.tile as tile
from concourse.bass_utils import run_bass_kernel_spmd

# Problem constants (hardcoded per contract)
B_FULL = 128
N_CORES = 8
B = B_FULL // N_CORES  # 16 samples per core
R = 2048
C = 10
O = 16
I = 8
ITERS = 3

RG = 8               # routes per matmul group
G = R // RG          # 256 groups
K = RG * I           # 64 contraction rows per group
CO = C * O           # 160
CH = 16              # groups per routing chunk
NCH = G // CH        # 16 chunks
CPAD = 12            # padded capsule dim for 4B alignment of bf16 rows

F32 = mybir.dt.float32
BF16 = mybir.dt.bfloat16

_COMPILED = None  # cache (nc, names) across calls


def _host_prep(x, W):
    """Build per-core DMA-ready arrays. x: (128,2048,8) W: (2048,10,16,8)."""
    x = np.ascontiguousarray(x, dtype=np.float32)
    W = np.ascontiguousarray(W, dtype=np.float32)

    # Wt[g, r_l*8+i, o*10+c] = W[g*8+r_l, c, o, i]
    Wt = W.transpose(0, 3, 2, 1).reshape(G, RG, I, O, C).reshape(G, K, CO)
    Wt = np.ascontiguousarray(Wt)

    # Bmask[(r_l,b), (r_l',b')] = 1 if b==b'  -> psum = sum over r_l,
    # replicated across all output partitions
    bmask = np.tile(np.eye(B, dtype=np.float32), (RG, RG))
    bmask_bf = bmask.astype(np.float32)  # used for bf16 rhs too (lhsT stays f32)

    shards = []
    for ci in range(N_CORES):
        xs = x[ci * B : (ci + 1) * B]  # (16, 2048, 8)
        # xt[g, r_l, i, b] = xs[b, g*8+r_l, i]
        xt = xs.transpose(1, 2, 0).reshape(G, RG, I, B)
        # Block-diagonal lhsT: Xblk[g, r_l*8+i, r_l*16+b] = xt[g, r_l, i, b]
        xblk = np.zeros((G, RG, I, RG, B), dtype=np.float32)
        idx = np.arange(RG)
        xblk[:, idx, :, idx, :] = xt.transpose(1, 0, 2, 3)
        shards.append(np.ascontiguousarray(xblk.reshape(G, K, RG * B)))
    return shards, Wt, bmask_bf


def _register_const(nc, dtype, value):
    t = nc.alloc_sbuf_tensor(f"const-{dtype.name}-{value}", [128, 1], dtype)
    nc.gpsimd.memset(t.ap(), value)
    nc.const_aps.aps[(dtype, value)] = t.ap()


def _build_kernel():
    nc = bacc.Bacc("TRN2", target_bir_lowering=False, debug=False,
                   num_devices=N_CORES)
    _register_const(nc, F32, 1e-8)
    nc.all_engine_barrier()

    xblk_d = nc.dram_tensor("xblk", [G, K, 128], F32, kind="ExternalInput")
    wt_d = nc.dram_tensor("wt", [G, K, CO], F32, kind="ExternalInput")
    bmask_d = nc.dram_tensor("bmask", [128, 128], F32, kind="ExternalInput")
    vout_d = nc.dram_tensor("vout", [B, O, C], F32, kind="ExternalOutput")

    with tile.TileContext(nc) as tc:
        with (
            tc.tile_pool(name="persist", bufs=1) as persist,
            tc.tile_pool(name="xw", bufs=2) as xw,
            tc.tile_pool(name="work", bufs=2) as work,
            tc.tile_pool(name="psum", bufs=4, space="PSUM") as psum,
            tc.tile_pool(name="spsum", bufs=2, space="PSUM") as spsum,
        ):
            uhat = persist.tile([128, G, O, C], BF16)      # 80 KiB/part
            bij = persist.tile([128, G, C], F32)           # 10 KiB
            cexp = persist.tile([128, G, C], F32)          # 10 KiB
            cbf = persist.tile([128, G, CPAD], BF16)       # 6 KiB
            bmask = persist.tile([128, 128], F32)
            bmask_b = persist.tile([128, 128], BF16)
            v_bf = persist.tile([128, O, C], BF16)
            s_sb = persist.tile([128, O, C], F32)
            sq = persist.tile([128, C], F32)
            sq2 = persist.tile([128, C], F32)
            zsum = persist.tile([128, G], F32)

            nc.sync.dma_start(bmask[:], bmask_d[:])
            nc.vector.tensor_copy(bmask_b[:], bmask[:])

            # ---------- Phase 1: u_hat build ----------
            DG = 8  # groups per DMA chunk
            for ch in range(G // DG):
                g0 = ch * DG
                xb = xw.tile([K, DG, 128], F32, tag="xb")
                wt = xw.tile([K, DG, CO], F32, tag="wt")
                nc.sync.dma_start(
                    xb[:], xblk_d[g0 : g0 + DG].rearrange("g k m -> k g m"))
                nc.sync.dma_start(
                    wt[:], wt_d[g0 : g0 + DG].rearrange("g k n -> k g n"))
                for j in range(DG // 2):
                    ps = psum.tile([128, 2, O, C], F32, tag="ps")
                    for t in range(2):
                        nc.tensor.matmul(
                            ps[:, t], lhsT=xb[:, j * 2 + t], rhs=wt[:, j * 2 + t],
                            start=True, stop=True)
                    g = g0 + j * 2
                    if j % 2 == 0:
                        nc.scalar.copy(uhat[:, g : g + 2], ps[:])
                    else:
                        nc.vector.tensor_copy(uhat[:, g : g + 2], ps[:])

            # ---------- helpers ----------
            def s_combine(sp, scale):
                # s_sb = (sp0 + sp1 + sp2) * scale; only one PSUM read per op
                nc.scalar.copy(s_sb[:], sp[:, 0])
                nc.vector.tensor_add(s_sb[:], s_sb[:], sp[:, 1])
                nc.vector.tensor_add(s_sb[:], s_sb[:], sp[:, 2])
                if scale != 1.0:
                    nc.scalar.mul(s_sb[:], s_sb[:], scale)

            # ---------- iteration 1 ----------
            # c == 0.1 exactly: s1 = 0.1 * sum_r u_hat, read u_hat directly.
            sp = spsum.tile([128, 3, O, C], F32, tag="sp")
            for m in range(G // 3):  # 85 triples + leftover group 255
                nc.tensor.matmul(
                    sp[:], lhsT=bmask_b[:],
                    rhs=uhat[:, 3 * m : 3 * m + 3].rearrange(
                        "p g o c -> p (g o c)"),
                    start=(m == 0), stop=False)
            nc.tensor.matmul(
                sp[:, 0], lhsT=bmask_b[:],
                rhs=uhat[:, 255].rearrange("p o c -> p (o c)"),
                start=False, stop=True)
            s_combine(sp, 0.1)
            _squash(nc, work, s_sb, sq, sq2, v_bf)

            # ---------- iterations 2..3 ----------
            for it in range(1, ITERS):
                # agreement pass -> bij
                for ch in range(NCH):
                    g0 = ch * CH
                    prod = work.tile([128, CH, O, C], BF16, tag="prod")
                    nc.vector.tensor_mul(
                        prod[:], uhat[:, g0 : g0 + CH],
                        v_bf[:].unsqueeze(1).broadcast_to((128, CH, O, C)))
                    nc.vector.tensor_add(prod[:, :, 0:8], prod[:, :, 0:8],
                                         prod[:, :, 8:16])
                    nc.vector.tensor_add(prod[:, :, 0:4], prod[:, :, 0:4],
                                         prod[:, :, 4:8])
                    nc.vector.tensor_add(prod[:, :, 0:2], prod[:, :, 0:2],
                                         prod[:, :, 2:4])
                    if it == 1:
                        nc.vector.tensor_add(bij[:, g0 : g0 + CH],
                                             prod[:, :, 0], prod[:, :, 1])
                    else:
                        nc.vector.tensor_add(prod[:, :, 0], prod[:, :, 0],
                                             prod[:, :, 1])
                        nc.vector.tensor_add(bij[:, g0 : g0 + CH],
                                             bij[:, g0 : g0 + CH], prod[:, :, 0])
                # softmax over c -> cbf
                nc.scalar.activation(cexp[:], bij[:],
                                     mybir.ActivationFunctionType.Exp)
                nc.vector.reduce_sum(zsum[:], cexp[:], axis=mybir.AxisListType.X)
                nc.vector.reciprocal(zsum[:], zsum[:])
                nc.vector.tensor_mul(
                    cbf[:, :, 0:C], cexp[:],
                    zsum[:].unsqueeze(2).broadcast_to((128, G, C)))
                # s pass: per chunk, multiply by c then sum group-triples on PE
                sp = spsum.tile([128, 3, O, C], F32, tag="sp")
                for ch in range(NCH):
                    g0 = ch * CH
                    prod = work.tile([128, CH, O, C], BF16, tag="prod")
                    nc.vector.tensor_mul(
                        prod[:], uhat[:, g0 : g0 + CH],
                        cbf[:, g0 : g0 + CH, 0:C].unsqueeze(2)
                        .broadcast_to((128, CH, O, C)))
                    # 5 triples + 1 single per 16-group chunk
                    for j in range(5):
                        nc.tensor.matmul(
                            sp[:], lhsT=bmask_b[:],
                            rhs=prod[:, 3 * j : 3 * j + 3].rearrange(
                                "p g o c -> p (g o c)"),
                            start=(ch == 0 and j == 0), stop=False)
                    nc.tensor.matmul(
                        sp[:, 0], lhsT=bmask_b[:],
                        rhs=prod[:, 15].rearrange("p o c -> p (o c)"),
                        start=False, stop=(ch == NCH - 1))
                s_combine(sp, 1.0)
                _squash(nc, work, s_sb, sq, sq2, v_bf)

            # ---------- output ----------
            vfin = work.tile([128, O, C], F32, tag="vfin")
            nc.vector.tensor_mul(
                vfin[:], s_sb[:],
                sq[:].unsqueeze(1).broadcast_to((128, O, C)))
            nc.sync.dma_start(vout_d[:], vfin[0:B])

    nc.compile()
    return nc


def _squash(nc, work, s_sb, sq, sq2, v_bf):
    """v = s * (|s|^2/(1+|s|^2)) / sqrt(|s|^2 + 1e-8), per (b, c).

    Leaves the final per-(p,c) scale factor in `sq`; v_bf = s * scale (bf16).
    s_sb layout (128, O, C); reduce over o via permuted view.
    """
    ssq = work.tile([128, O, C], F32, tag="ssq")
    nc.vector.tensor_mul(ssq[:], s_sb[:], s_sb[:])
    nc.vector.reduce_sum(sq[:], ssq[:].rearrange("p o c -> p c o"),
                         axis=mybir.AxisListType.X)
    # sq2 = (1+n)*sqrt(n+1e-8);  sq = n / sq2
    t1 = work.tile([128, C], F32, tag="sqt")
    nc.scalar.activation(sq2[:], sq[:], mybir.ActivationFunctionType.Sqrt,
                         bias=1e-8)
    nc.scalar.add(t1[:], sq[:], 1.0)
    nc.vector.tensor_mul(sq2[:], sq2[:], t1[:])
    nc.vector.reciprocal(sq2[:], sq2[:])
    nc.vector.tensor_mul(sq[:], sq[:], sq2[:])
    nc.vector.tensor_mul(
        v_bf[:], s_sb[:], sq[:].unsqueeze(1).broadcast_to((128, O, C)))


def kernel(x, W):
    global _COMPILED
    xshards, Wt, bmask = _host_prep(x, W)
    if _COMPILED is None:
        _COMPILED = _build_kernel()
    nc = _COMPILED
    in_maps = [
        {"xblk": xs, "wt": Wt, "bmask": bmask} for xs in xshards
    ]
    res = run_bass_kernel_spmd(nc, in_maps, list(range(N_CORES)))
    outs = []
    for ci in range(N_CORES):
        v = res.results[ci]["vout"]  # (16, O, C)
        outs.append(v.transpose(0, 2, 1))  # -> (16, C, O)
    return np.ascontiguousarray(np.concatenate(outs, axis=0), dtype=np.float32)


# revision 19
# speedup vs baseline: 16.7697x; 16.7697x over previous
"""DigitCaps dynamic-routing kernel for 8x Trainium2 NeuronCores.

Full inputs -> batch-sharded across 8 cores (16 samples/core), W replicated.

Per-core layout:
  u_hat[(r_l,b)=128 partitions, g=256, o=16, c=10]  (bf16 in SBUF)
    where route r = g*8 + r_l   (8 routes per matmul group)
  b_ij / c_ij: (128p, 256g, 10c)   (partition carries (r_l, b))

u_hat build: per group g, one PE matmul:
  lhsT = Xblk[g] (K=64=(r_l,i), M=128=(r_l,b))  block-diagonal x (host-built)
  rhs  = Wt[g]   (K=64=(r_l,i), N=160=(o,c))
  out  = psum (128=(r_l,b), 160=(o,c)) -> drain to SBUF as bf16

s_j = sum_r c*u_hat: PE matmuls with constant lhsT = tile(eye(16),(8,8)):
  sums over partitions (r_l) while replicating the (16b, 160) result to all
  128 partitions; groups are summed 3-at-a-time into 3 psum slots
  (accumulated over chunks) + a final 2-op add.

agreement = sum_o u_hat*v: DVE broadcast multiply + tree-reduce over o.
softmax over c: free-dim innermost; squash: tiny (128,160) ops.
"""

import sys

for p in ("/opt/trn_rl_repo",):
    if p not in sys.path:
        sys.path.insert(0, p)

import numpy as np

import concourse.bass as bass
import concourse.bacc as bacc
import concourse.mybir as mybir
import concourse.tile as tile
from concourse.bass_utils import run_bass_kernel_spmd

# Problem constants (hardcoded per contract)
B_FULL = 128
N_CORES = 8
B = B_FULL // N_CORES  # 16 samples per core
R = 2048
C = 10
O = 16
I = 8
ITERS = 3

RG = 8               # routes per matmul group
G = R // RG          # 256 groups
K = RG * I           # 64 contraction rows per group
CO = C * O           # 160
CH = 16              # groups per routing chunk
NCH = G // CH        # 16 chunks
CPAD = 12            # padded capsule dim for 4B alignment of bf16 rows

F32 = mybir.dt.float32
BF16 = mybir.dt.bfloat16

_COMPILED = None  # cache (nc, names) across calls


def _host_prep(x, W):
    """Build per-core DMA-ready arrays. x: (128,2048,8) W: (2048,10,16,8)."""
    x = np.ascontiguousarray(x, dtype=np.float32)
    W = np.ascontiguousarray(W, dtype=np.float32)

    # Wt[g, r_l*8+i, o*10+c] = W[g*8+r_l, c, o, i]
    Wt = W.transpose(0, 3, 2, 1).reshape(G, RG, I, O, C).reshape(G, K, CO)
    # interleave for the build layout: chunk ch of 8 groups; group
    # g0+two*4+gp -> partitions two*64..+63, free slot gp
    Wt = np.ascontiguousarray(
        Wt.reshape(G // 8, 2, 4, K, CO).transpose(0, 1, 3, 2, 4)
        .reshape(G // 8, 128, 4, CO))

    # Bmask[(r_l,b), (r_l',b')] = 1 if b==b'  -> psum = sum over r_l,
    # replicated across all output partitions
    bmask = np.tile(np.eye(B, dtype=np.float32), (RG, RG))
    bmask_bf = bmask.astype(np.float32)  # used for bf16 rhs too (lhsT stays f32)

    shards = []
    for ci in range(N_CORES):
        xs = x[ci * B : (ci + 1) * B]  # (16, 2048, 8)
        # xt[g, r_l, i, b] = xs[b, g*8+r_l, i]
        xt = xs.transpose(1, 2, 0).reshape(G, RG, I, B)
        # Block-diagonal lhsT: Xblk[g, r_l*8+i, r_l*16+b] = xt[g, r_l, i, b]
        xblk = np.zeros((G, RG, I, RG, B), dtype=np.float32)
        idx = np.arange(RG)
        xblk[:, idx, :, idx, :] = xt.transpose(1, 0, 2, 3)
        xblk = xblk.reshape(G, K, RG * B)
        xblk = np.ascontiguousarray(
            xblk.reshape(G // 8, 2, 4, K, RG * B).transpose(0, 1, 3, 2, 4)
            .reshape(G // 8, 128, 4, RG * B))
        shards.append(xblk)
    return shards, Wt, bmask_bf


def _register_const(nc, dtype, value):
    t = nc.alloc_sbuf_tensor(f"const-{dtype.name}-{value}", [128, 1], dtype)
    nc.gpsimd.memset(t.ap(), value)
    nc.const_aps.aps[(dtype, value)] = t.ap()


def _build_kernel():
    nc = bacc.Bacc("TRN2", target_bir_lowering=False, debug=False,
                   num_devices=N_CORES)
    _register_const(nc, F32, 1e-8)
    nc.all_engine_barrier()

    xblk_d = nc.dram_tensor("xblk", [G // 8, 128, 4, 128], F32,
                            kind="ExternalInput")
    wt_d = nc.dram_tensor("wt", [G // 8, 128, 4, CO], F32,
                          kind="ExternalInput")
    bmask_d = nc.dram_tensor("bmask", [128, 128], F32, kind="ExternalInput")
    vout_d = nc.dram_tensor("vout", [B, O, C], F32, kind="ExternalOutput")

    with tile.TileContext(nc) as tc:
        with (
            tc.tile_pool(name="persist", bufs=1) as persist,
            tc.tile_pool(name="xw", bufs=2) as xw,
            tc.tile_pool(name="work", bufs=2) as work,
            tc.tile_pool(name="psum", bufs=6, space="PSUM") as psum,
            tc.tile_pool(name="spsum", bufs=2, space="PSUM") as spsum,
        ):
            uhat = persist.tile([128, G, O, C], BF16)      # 80 KiB/part
            bij = persist.tile([128, G, C], F32)           # 10 KiB
            cexp = persist.tile([128, G, C], F32)          # 10 KiB
            cbf = persist.tile([128, G, CPAD], BF16)       # 6 KiB
            bmask = persist.tile([128, 128], F32)
            bmask_b = persist.tile([128, 128], BF16)
            v_bf = persist.tile([128, O, C], BF16)
            s_sb = persist.tile([128, O, C], F32)
            sq = persist.tile([128, C], F32)
            sq2 = persist.tile([128, C], F32)
            zsum = persist.tile([128, G], F32)

            nc.sync.dma_start(bmask[:], bmask_d[:])
            nc.vector.tensor_copy(bmask_b[:], bmask[:])

            # ---------- Phase 1: u_hat build ----------
            # Group pairs (g0+j, g0+4+j) run concurrently in PE row-groups
            # 0-63 / 64-127 (K=64 each); their outputs go to different PSUM
            # banks so the row-tiles don't serialize on the bank tracker.
            DG = 8  # groups per DMA chunk
            for ch in range(G // DG):
                g0 = ch * DG
                # partition (two*64+k) holds group g0+gp*2+two... see rearrange
                xb = xw.tile([128, DG // 2, 128], F32, tag="xb")
                wt = xw.tile([128, DG // 2, CO], F32, tag="wt")
                # dest[(two k), gp, m] = src[(two gp), k, m]:
                #   group g0+two*4+gp -> partitions two*64..two*64+63, slot gp
                nc.sync.dma_start(xb[:], xblk_d[ch])
                nc.scalar.dma_start(wt[:], wt_d[ch])
                # psum tiles: T0=(g0,g0+1) T1=(g0+2,g0+3) T2=(g0+4,..) T3=...
                # concurrent row-group pair (g0+j, g0+4+j) lands in different
                # tiles (T[j//2] vs T[2+j//2]) -> different banks
                pst = [psum.tile([128, 2, O, C], F32, tag="ps", name=f"ps{ch}_{t}")
                       for t in range(4)]
                for j in range(4):
                    for two in range(2):
                        g = g0 + two * 4 + j
                        lo = (g - g0) % DG
                        nc.tensor.matmul(
                            pst[lo // 2][:, lo & 1],
                            lhsT=xb[two * 64 : two * 64 + 64, j],
                            rhs=wt[two * 64 : two * 64 + 64, j],
                            start=True, stop=True)
                for t in range(4):
                    nc.scalar.copy(uhat[:, g0 + 2 * t : g0 + 2 * t + 2], pst[t][:])

            # ---------- helpers ----------
            def s_combine(sp, scale):
                # s_sb = (sp0 + sp1 + sp2) * scale; only one PSUM read per op
                nc.scalar.copy(s_sb[:], sp[:, 0])
                nc.vector.tensor_add(s_sb[:], s_sb[:], sp[:, 1])
                nc.vector.tensor_add(s_sb[:], s_sb[:], sp[:, 2])
                if scale != 1.0:
                    nc.scalar.mul(s_sb[:], s_sb[:], scale)

            # ---------- iteration 1 ----------
            # c == 0.1 exactly: s1 = 0.1 * sum_r u_hat, read u_hat directly.
            sp = spsum.tile([128, 3, O, C], F32, tag="sp")
            for m in range(G // 3):  # 85 triples + leftover group 255
                nc.tensor.matmul(
                    sp[:], lhsT=bmask_b[:],
                    rhs=uhat[:, 3 * m : 3 * m + 3].rearrange(
                        "p g o c -> p (g o c)"),
                    start=(m == 0), stop=False)
            nc.tensor.matmul(
                sp[:, 0], lhsT=bmask_b[:],
                rhs=uhat[:, 255].rearrange("p o c -> p (o c)"),
                start=False, stop=True)
            s_combine(sp, 0.1)
            _squash(nc, work, s_sb, sq, sq2, v_bf)

            # ---------- iterations 2..3 ----------
            for it in range(1, ITERS):
                # agreement pass -> bij
                for ch in range(NCH):
                    g0 = ch * CH
                    prod = work.tile([128, CH, O, C], BF16, tag="prod")
                    nc.vector.tensor_mul(
                        prod[:], uhat[:, g0 : g0 + CH],
                        v_bf[:].unsqueeze(1).broadcast_to((128, CH, O, C)))
                    nc.vector.tensor_add(prod[:, :, 0:8], prod[:, :, 0:8],
                                         prod[:, :, 8:16])
                    nc.vector.tensor_add(prod[:, :, 0:4], prod[:, :, 0:4],
                                         prod[:, :, 4:8])
                    nc.vector.tensor_add(prod[:, :, 0:2], prod[:, :, 0:2],
                                         prod[:, :, 2:4])
                    if it == 1:
                        nc.vector.tensor_add(bij[:, g0 : g0 + CH],
                                             prod[:, :, 0], prod[:, :, 1])
                    else:
                        nc.vector.tensor_add(prod[:, :, 0], prod[:, :, 0],
                                             prod[:, :, 1])
                        nc.vector.tensor_add(bij[:, g0 : g0 + CH],
                                             bij[:, g0 : g0 + CH], prod[:, :, 0])
                # softmax over c -> cbf
                nc.scalar.activation(cexp[:], bij[:],
                                     mybir.ActivationFunctionType.Exp)
                nc.vector.reduce_sum(zsum[:], cexp[:], axis=mybir.AxisListType.X)
                nc.vector.reciprocal(zsum[:], zsum[:])
                nc.vector.tensor_mul(
                    cbf[:, :, 0:C], cexp[:],
                    zsum[:].unsqueeze(2).broadcast_to((128, G, C)))
                # s pass: per chunk, multiply by c then sum group-triples on PE
                sp = spsum.tile([128, 3, O, C], F32, tag="sp")
                for ch in range(NCH):
                    g0 = ch * CH
                    prod = work.tile([128, CH, O, C], BF16, tag="prod")
                    nc.vector.tensor_mul(
                        prod[:], uhat[:, g0 : g0 + CH],
                        cbf[:, g0 : g0 + CH, 0:C].unsqueeze(2)
                        .broadcast_to((128, CH, O, C)))
                    # 5 triples + 1 single per 16-group chunk
                    for j in range(5):
                        nc.tensor.matmul(
                            sp[:], lhsT=bmask_b[:],
                            rhs=prod[:, 3 * j : 3 * j + 3].rearrange(
                                "p g o c -> p (g o c)"),
                            start=(ch == 0 and j == 0), stop=False)
                    nc.tensor.matmul(
                        sp[:, 0], lhsT=bmask_b[:],
                        rhs=prod[:, 15].rearrange("p o c -> p (o c)"),
                        start=False, stop=(ch == NCH - 1))
                s_combine(sp, 1.0)
                _squash(nc, work, s_sb, sq, sq2, v_bf)

            # ---------- output ----------
            vfin = work.tile([128, O, C], F32, tag="vfin")
            nc.vector.tensor_mul(
                vfin[:], s_sb[:],
                sq[:].unsqueeze(1).broadcast_to((128, O, C)))
            nc.sync.dma_start(vout_d[:], vfin[0:B])

    nc.compile()
    return nc


def _squash(nc, work, s_sb, sq, sq2, v_bf):
    """v = s * (|s|^2/(1+|s|^2)) / sqrt(|s|^2 + 1e-8), per (b, c).

    Leaves the final per-(p,c) scale factor in `sq`; v_bf = s * scale (bf16).
    s_sb layout (128, O, C); reduce over o via permuted view.
    """
    ssq = work.tile([128, O, C], F32, tag="ssq")
    nc.vector.tensor_mul(ssq[:], s_sb[:], s_sb[:])
    nc.vector.reduce_sum(sq[:], ssq[:].rearrange("p o c -> p c o"),
                         axis=mybir.AxisListType.X)
    # sq2 = (1+n)*sqrt(n+1e-8);  sq = n / sq2
    t1 = work.tile([128, C], F32, tag="sqt")
    nc.scalar.activation(sq2[:], sq[:], mybir.ActivationFunctionType.Sqrt,
                         bias=1e-8)
    nc.scalar.add(t1[:], sq[:], 1.0)
    nc.vector.tensor_mul(sq2[:], sq2[:], t1[:])
    nc.vector.reciprocal(sq2[:], sq2[:])
    nc.vector.tensor_mul(sq[:], sq[:], sq2[:])
    nc.vector.tensor_mul(
        v_bf[:], s_sb[:], sq[:].unsqueeze(1).broadcast_to((128, O, C)))


def kernel(x, W):
    global _COMPILED
    xshards, Wt, bmask = _host_prep(x, W)
    if _COMPILED is None:
        _COMPILED = _build_kernel()
    nc = _COMPILED
    in_maps = [
        {"xblk": xs, "wt": Wt, "bmask": bmask} for xs in xshards
    ]
    res = run_bass_kernel_spmd(nc, in_maps, list(range(N_CORES)))
    outs = []
    for ci in range(N_CORES):
        v = res.results[ci]["vout"]  # (16, O, C)
        outs.append(v.transpose(0, 2, 1))  # -> (16, C, O)
    return np.ascontiguousarray(np.concatenate(outs, axis=0), dtype=np.float32)
